# revision 35
# baseline (speedup 1.0000x reference)
"""Trainium2 Bass kernel for an RNN-T joint network.

Computation (per batch element b):
    enc_p  = enc_out @ W_enc + b_enc                      # (T, J)
    pred_p = pred_out @ W_pred + b_pred                   # (U, J)
    joint  = tanh(enc_p[:, None, :] + pred_p[None, :, :]) # (T, U, J)
    logits = joint @ W_joint + b_joint                    # (T, U, V)

Distribution: data-parallel over the batch dim B=8, one batch element per
NeuronCore.  J-major layout (J on the SBUF partition dim) so the broadcast
add + tanh fuses into one ScalarE activation per (j-tile, u) and the vocab
matmul contracts over the partition dim.

Key optimizations (steady state measures ~225us/iter vs the ~218us bf16
tensor-engine roofline for the 17.2 GFLOP/core vocab projection):
  * inputs are pre-transposed and pre-cast to bf16 on the host (enc/pred
    arrive as (D, T)/(D, U); weights as bf16) - no PE transposes, no
    on-device casts, half the input DMA bytes.  b_enc+b_pred summed on host.
  * logits stored as bf16 (halves the 64 MiB/core store traffic; host
    upcasts to fp32).  bf16 math keeps L2 rel err ~4e-3, far under the
    2e-2 gate.
  * stores batched 4 u's per DMA -> stores of 2 MiB with 8 KiB contiguous
    DRAM chunks; the last group is split so the unhidden final store is
    small.
  * PSUM as [128, 1024] two-bank tiles; one VectorE drain (bias add,
    fp32->bf16) per (u, t-half).
  * inputs packed on the host into partition-major blobs laid out exactly
    as SBUF wants them, so each load DMA moves one large contiguous chunk
    per partition (128 descriptors/DMA; DMA issue costs ~8ns/descriptor,
    so fine-grained loads serialized ~20us of issue on the Pool engine).
    Loads ride two queues; W_joint is split per j-tile so the first vocab
    matmul starts as soon as wj[0] lands.
  * each joint tile is consumed in one 4-matmul burst so ScalarE tanh
    production stays ahead of PE demand.
"""

from contextlib import ExitStack

import numpy as np

import concourse.bacc as bacc
import concourse.mybir as mybir
import concourse.tile as tile
import concourse.bass_utils as bass_utils

N_CORES = 8
T, U, J, V = 256, 64, 512, 1024
DE, DP = 512, 640
KJ = J // 128    # j-tiles
KE = DE // 128   # enc contraction tiles
KP = DP // 128   # pred contraction tiles
HT = T // 128    # t-halves
F32 = mybir.dt.float32
BF16 = mybir.dt.bfloat16

# Offsets (in elements, per partition) into the packed bf16 input blob.
# The host lays inputs out partition-major so every load DMA moves one
# large contiguous chunk per partition (128 descriptors per DMA instead
# of thousands of 512B-2KB ones - DMA issue costs ~8ns/descriptor).
# W_enc/W_pred are packed by j-block so the j=0 slices sit right after
# the activations: the first load DMA carries exactly what stage-1's
# j=0 matmuls need.
OFF_ENC = 0                      # [KE, T]       enc_out^T
OFF_WENC = OFF_ENC + KE * T      # [KJ, KE, 128] W_enc, j-major
OFF_PRED = OFF_WENC + KE * J     # [KP, U]       pred_out^T
OFF_WPRED = OFF_PRED + KP * U    # [KJ, KP, 128] W_pred, j-major
OFF_WJ = OFF_WPRED + KP * J      # [KJ, V]       W_joint
NB16 = OFF_WJ + KJ * V
NB32 = KJ + V                    # [KJ] b_enc+b_pred | [V] b_joint (replicated)

_CACHE: dict = {}
# Ablation switches for bench/devloop only.
_OPTS: dict = {"no_store": False, "no_mm2": False, "no_act": False, "no_drain": False,
               "wide_mm": False,  # N=1024 matmul fails the s3d3 ISA check (PSUM bank)
               "prewarm": 0,
               "group": 4, "ps_bufs": 4, "joint_bufs": 12, "out_bufs": 2}


def _emit(tc, nc, d, repeats=1):
    act = mybir.ActivationFunctionType
    g = _OPTS["group"]
    ng = U // g
    with ExitStack() as ctx:
        const = ctx.enter_context(tc.tile_pool(name="const", bufs=1))
        stg = ctx.enter_context(tc.tile_pool(name="stg", bufs=1 if repeats == 1 else 2))
        joint_pool = ctx.enter_context(tc.tile_pool(name="jp", bufs=_OPTS["joint_bufs"]))
        out_pool = ctx.enter_context(tc.tile_pool(name="op", bufs=_OPTS["out_bufs"]))

        # ---------------- loads: packed blobs, two queues ----------------
        # gpsimd queue: stage-1 inputs + biases; sync queue: pred inputs +
        # W_joint (split per j so the first mm2 starts when wj[0] lands).
        mega16 = const.tile([128, NB16], BF16, tag="mega16")
        mega32 = const.tile([128, NB32], F32, tag="mega32")
        b16 = d["blob16"].ap()
        # gpsimd queue: bsum (tiny), enc + W_enc[j=0] prefix, W_enc[j>0],
        # W_pred[j>0], bj broadcast.  sync queue: pred + W_pred[j=0], then
        # W_joint per j-tile.  Ordered so each consumer's input lands just
        # before it is needed on the independent HW queues.
        b32 = d["blob32"].ap()
        nc.gpsimd.dma_start(mega32[:, 0:KJ], b32[:, 0:KJ])
        cut_e = OFF_WENC + KE * 128
        cut_p = OFF_WPRED + KP * 128
        nc.gpsimd.dma_start(mega16[:, OFF_ENC:cut_e], b16[:, OFF_ENC:cut_e])
        nc.gpsimd.dma_start(mega16[:, cut_e:OFF_PRED], b16[:, cut_e:OFF_PRED])
        nc.gpsimd.dma_start(mega16[:, cut_p:OFF_WJ], b16[:, cut_p:OFF_WJ])
        nc.gpsimd.dma_start(mega32[:, KJ:NB32], b32[:, KJ:NB32])
        nc.sync.dma_start(mega16[:, OFF_PRED:cut_p], b16[:, OFF_PRED:cut_p])
        for j in range(KJ):
            o = OFF_WJ + j * V
            nc.sync.dma_start(mega16[:, o:o + V], b16[:, o:o + V])

        def enc_t(k):
            return mega16[:, OFF_ENC + k * T:OFF_ENC + (k + 1) * T]

        def wenc(k, j):
            o = OFF_WENC + j * (KE * 128) + k * 128
            return mega16[:, o:o + 128]

        def pred_t(k):
            return mega16[:, OFF_PRED + k * U:OFF_PRED + (k + 1) * U]

        def wpred(k, j):
            o = OFF_WPRED + j * (KP * 128) + k * 128
            return mega16[:, o:o + 128]

        def wj(j, v0, v1):
            o = OFF_WJ + j * V
            return mega16[:, o + v0:o + v1]

        bj_sb = mega32[:, KJ:KJ + V]

        if _OPTS["prewarm"]:
            # Keep PE busy during the input-DMA phase so the HAM clock gate
            # un-throttles (K=8/8) before the real matmul stream begins.
            junk = const.tile([128, 640], BF16, tag="pw_junk")
            nc.gpsimd.memset(junk[:], 0.0)
            with tc.tile_pool(name="pw_ps", bufs=1, space="PSUM") as pwp:
                pw = pwp.tile([128, 512], F32, tag="pw")
                for _ in range(_OPTS["prewarm"]):
                    nc.tensor.matmul(pw[:], junk[:, :128], junk[:, 128:640],
                                     start=True, stop=True)

        out_ap = d["logits"].ap()
        for rep in range(repeats):
            with tc.tile_pool(name=f"ps1_{rep}", bufs=2, space="PSUM") as ps1:
                # ---------------- stage-1 projections ----------------
                # j-outer: enc_p[0] / pred_p[0] complete early so the
                # ScalarE tanh pipeline starts filling immediately.
                # Interleave pred after the first enc j so ACT(u=0) can
                # begin as soon as (enc_p[0], pred_p[0]) exist.
                enc_p = stg.tile([128, KJ, T], F32, tag="enc_p", name=f"enc_p_{rep}")
                pred_p = stg.tile([128, KJ, U], F32, tag="pred_p", name=f"pred_p_{rep}")
                for j in range(KJ):
                    pt = ps1.tile([128, T], F32, tag="ps1mm", name=f"mm_enc_{rep}_{j}")
                    for k in range(KE):
                        nc.tensor.matmul(
                            pt[:],
                            wenc(k, j),
                            enc_t(k),
                            start=(k == 0),
                            stop=(k == KE - 1),
                        )
                    nc.vector.tensor_copy(enc_p[:, j, :], pt[:])
                    # pred_p[j] right after enc_p[j]; carries b_enc + b_pred.
                    pp = ps1.tile([128, U], F32, tag="ps1pu", name=f"mm_pred_{rep}_{j}")
                    for k in range(KP):
                        nc.tensor.matmul(
                            pp[:],
                            wpred(k, j),
                            pred_t(k),
                            start=(k == 0),
                            stop=(k == KP - 1),
                        )
                    nc.vector.tensor_scalar_add(pred_p[:, j, :], pp[:], mega32[:, j:j + 1])

            # ---------------- main loop over u, grouped stores ----------
            # Tapered tail: the last group is split 2+1+1 and the final
            # stores alternate DMA queues, so the unhidden tail after the
            # last matmul is one small parallel store pair.
            groups = [g] * (ng - 1) + [2, 1, 1] if g == 4 else [g] * ng
            assert sum(groups) == U
            with tc.tile_pool(name=f"ps2_{rep}", bufs=_OPTS["ps_bufs"], space="PSUM") as ps2:
                u_base = 0
                for gi, gsz in enumerate(groups):
                    ot = out_pool.tile([128, HT, gsz * V], BF16,
                                       tag=f"out{gsz}", name=f"out_{rep}_{gi}")
                    for ui in range(gsz):
                        u = u_base + ui
                        jt = []
                        for j in range(KJ):
                            jtile = joint_pool.tile([128, T], BF16, tag="joint",
                                                    name=f"joint_{rep}_{u}_{j}")
                            if not _OPTS["no_act"]:
                                nc.scalar.activation(
                                    jtile[:], enc_p[:, j, :], act.Tanh,
                                    bias=pred_p[:, j, u:u + 1], scale=1.0,
                                )
                            else:
                                nc.gpsimd.memset(jtile[:, :1], 0.0)
                            jt.append(jtile)
                        # j-outer, h-inner: each joint tile jt[j] is consumed
                        # in one burst, so ScalarE only has to deliver one
                        # tile per ~0.9us of PE work (helps the early ramp).
                        pts = [ps2.tile([128, V], F32, tag="ps2", name=f"mm_{rep}_{u}_{h}")
                               for h in range(HT)]
                        if not _OPTS["no_mm2"]:
                            for j in range(KJ):
                                for h in range(HT):
                                    for vh in range(2):
                                        nc.tensor.matmul(
                                            pts[h][:, vh * 512:(vh + 1) * 512],
                                            jt[j][:, h * 128:(h + 1) * 128],
                                            wj(j, vh * 512, (vh + 1) * 512),
                                            start=(j == 0),
                                            stop=(j == KJ - 1),
                                        )
                        else:
                            for h in range(HT):
                                for vh in range(2):
                                    nc.tensor.matmul(
                                        pts[h][:, vh * 512:(vh + 1) * 512],
                                        jt[0][:, h * 128:(h + 1) * 128],
                                        wj(0, vh * 512, (vh + 1) * 512),
                                        start=True, stop=True,
                                    )
                        for h in range(HT):
                            if not _OPTS["no_drain"]:
                                nc.vector.tensor_tensor(
                                    ot[:, h, ui * V:(ui + 1) * V],
                                    pts[h][:],
                                    bj_sb,
                                    mybir.AluOpType.add,
                                )
                            else:
                                nc.vector.tensor_copy(ot[:, h, ui * V:ui * V + 1], pts[h][:, :1])
                    if not _OPTS["no_store"]:
                        # Last store rides the (idle) gpsimd queue so the
                        # final two stores drain in parallel.
                        eng = nc.gpsimd if gi == len(groups) - 1 else nc.sync
                        eng.dma_start(
                            out_ap[:, u_base:u_base + gsz, :].rearrange(
                                "(h p) u v -> p h (u v)", p=128),
                            ot[:],
                        )
                    u_base += gsz


def _build_program(repeats=1):
    nc = bacc.Bacc("TRN2", target_bir_lowering=False, debug=False, num_devices=N_CORES)
    d = {
        "blob16": nc.dram_tensor("blob16", (128, NB16), BF16, kind="ExternalInput"),
        "blob32": nc.dram_tensor("blob32", (128, NB32), F32, kind="ExternalInput"),
        "logits": nc.dram_tensor("logits", (T, U, V), BF16, kind="ExternalOutput"),
    }
    with tile.TileContext(nc) as tc:
        _emit(tc, nc, d, repeats=repeats)
    nc.compile()
    return nc


def _pack16(arr, k):
    """(k*128, N) -> (128, k*N): partition p holds rows p, 128+p, ..."""
    n = arr.shape[1]
    return arr.reshape(k, 128, n).transpose(1, 0, 2).reshape(128, k * n)


def _pack16_jmajor(arr, k):
    """(k*128, KJ*128) -> (128, KJ*k*128): j-block-major weight packing.
    Element [p, j*(k*128) + ki*128 + m] = arr[ki*128 + p, j*128 + m]."""
    return (arr.reshape(k, 128, KJ, 128).transpose(1, 2, 0, 3)
            .reshape(128, KJ * k * 128))


def _prep_in_maps(enc_out, pred_out, W_enc, b_enc, W_pred, b_pred, W_joint, b_joint):
    bf = mybir.dt.np(BF16)
    wenc_p = _pack16_jmajor(np.asarray(W_enc, dtype=np.float32).astype(bf), KE)
    wpred_p = _pack16_jmajor(np.asarray(W_pred, dtype=np.float32).astype(bf), KP)
    wj_p = _pack16(np.asarray(W_joint, dtype=np.float32).astype(bf), KJ)
    bsum = (np.asarray(b_enc, dtype=np.float32)
            + np.asarray(b_pred, dtype=np.float32)).reshape(KJ, 128).T
    bj = np.broadcast_to(np.asarray(b_joint, dtype=np.float32), (128, V))
    blob32 = np.ascontiguousarray(
        np.concatenate([bsum, bj], axis=1), dtype=np.float32)
    in_maps = []
    for c in range(N_CORES):
        enc_p = _pack16(np.asarray(enc_out[c], dtype=np.float32).astype(bf).T, KE)
        pred_p = _pack16(np.asarray(pred_out[c], dtype=np.float32).astype(bf).T, KP)
        blob16 = np.ascontiguousarray(
            np.concatenate([enc_p, wenc_p, pred_p, wpred_p, wj_p], axis=1))
        in_maps.append({"blob16": blob16, "blob32": blob32})
    return in_maps


def kernel(enc_out, pred_out, W_enc, b_enc, W_pred, b_pred, W_joint, b_joint):
    nc = _CACHE.get("nc")
    if nc is None:
        nc = _CACHE["nc"] = _build_program()
    in_maps = _prep_in_maps(enc_out, pred_out, W_enc, b_enc, W_pred, b_pred,
                            W_joint, b_joint)
    res = bass_utils.run_bass_kernel_spmd(nc, in_maps, core_ids=list(range(N_CORES)))
    _CACHE["last_results"] = res
    return np.stack([np.asarray(res.results[c]["logits"]).astype(np.float32)
                     for c in range(N_CORES)])
